# revision 7
# baseline (speedup 1.0000x reference)
"""Euclidean distance matrix (torch.cdist p=2) on 8 Trainium2 NeuronCores.

Strategy (data-parallel over x1 rows, per the sharding hint):
  - Shard x1 rows 8 ways; replicate x2. Each core computes a [1024, 8192]
    tile of the output distance matrix.
  - d2 = ||a||^2 + ||b||^2 - 2 a.b:
      * cross term as fp16 matmuls with K = 256 (two K=128 phases),
      * ||b||^2 via a K=2 aug matmul (sq2 split hi/lo fp16 vs a ones
        weight), packed 4-to-the-array via row-group tile_position,
      * ||a||^2 via the ACT engine's per-partition bias operand folded
        into the Sqrt eviction: out = sqrt(psum + sq1[row]).
  - PSUM is 2 ping-pong tiles of [128, 2048] (4 banks each): the PE fills
    one while one Scalar ACTIVATE (Sqrt+bias) evicts the other to SBUF
    staging in a single instruction. 32 evictions total.
  - Inputs stream in column chunks ordered first-needed-first on the sync
    (HWDGE, FIFO) ring, so matmuls start as soon as ~1.1 MB has landed and
    the m=0 block trickle-computes behind the input stream.
  - Output staging is fp16; the SWDGE output DMA casts fp16 -> fp32 in the
    DMA datapath, halving the SBUF-side read bytes of the output stream
    (which otherwise paces the whole kernel at ~27 GB/s x 16 engines).
    One 1 MB (fp32-side) DMA per psum tile, 32 total, starting as soon as
    the first psum tile is evicted.
  - LDWEIGHTS dedupe post-pass as in the baseline: one stationary load
    feeds 4 consecutive matmuls.
"""

import numpy as np

N1 = 8192  # x1 rows (output rows)
N2 = 8192  # x2 rows (output cols)
D = 256    # feature dim
NCORES = 8
M1 = N1 // NCORES  # 1024 output rows per core
P = 128            # partitions
NT = 512           # matmul moving free dim
PW = 2048          # psum tile width (4 banks); 2 bufs = full PSUM
NCHUNK = N2 // PW  # 4 column chunks
AUG = 2            # aug rows carrying the sq2 hi/lo terms
OBUFS = 6          # output staging buffers

_built = None


def _ldw_key(inst):
    ap = inst.ins[0]
    return str(ap)


def _dedupe_ldweights(nc):
    """Drop InstLdweights whose weights AP equals the currently-loaded one
    (no different load in between on the PE stream). Their rare sync waits
    are migrated to the next PE instruction; Bacc.finalize() later splits
    any resulting multi-wait into EventSemaphore preludes."""
    import concourse.mybir as mybir

    dropped = 0
    for f in nc.m.functions:
        for blk in f.blocks:
            insts = list(blk.instructions)
            cur_key = None
            pending = []
            to_drop = []
            for inst in insts:
                if isinstance(inst, mybir.InstLdweights):
                    key = _ldw_key(inst)
                    if key == cur_key:
                        si = inst.sync_info
                        if si is not None and si.on_wait:
                            pending.extend(si.on_wait)
                        to_drop.append(inst)
                    else:
                        cur_key = key
                elif isinstance(inst, mybir.InstMatmult):
                    if pending:
                        si = inst.sync_info
                        waits = list(si.on_wait) if si else []
                        upds = list(si.on_update) if si else []
                        inst.sync_info = mybir.SyncInfo(
                            on_wait=waits + pending, on_update=upds
                        )
                        pending = []
            assert not pending
            for inst in to_drop:
                blk.instructions.remove(inst)
            dropped += len(to_drop)
    return dropped


def _build_nc():
    import concourse.bass as bass
    import concourse.mybir as mybir
    from concourse import bacc, tile

    f16 = mybir.dt.float16
    f32 = mybir.dt.float32

    nc = bacc.Bacc(None, target_bir_lowering=False)
    a_feat = nc.declare_dram_parameter("a_feat", [D, M1], f16, isOutput=False)
    sq1t = nc.declare_dram_parameter("sq1t", [P, M1 // P], f32, isOutput=False)
    b_feat = nc.declare_dram_parameter("b_feat", [D, N2], f16, isOutput=False)
    b_aug = nc.declare_dram_parameter("b_aug", [AUG, N2], f16, isOutput=False)
    out = nc.declare_dram_parameter("out", [M1, N2], f32, isOutput=True)

    Sqrt = mybir.ActivationFunctionType.Sqrt

    with tile.TileContext(nc) as tc:
        with (
            tc.tile_pool(name="persist", bufs=1) as persist,
            tc.tile_pool(name="ostage", bufs=OBUFS) as ostage,
            tc.tile_pool(name="ps", bufs=2, space=bass.MemorySpace.PSUM) as pspool,
        ):
            a0 = persist.tile([P, M1], f16, tag="a0")
            a1 = persist.tile([P, M1], f16, tag="a1")
            ones = persist.tile([P, P], f16, tag="ones")
            sq1s = persist.tile([P, M1 // P], f32, tag="sq1s")
            baug = persist.tile([P, N2], f16, tag="baug")
            b = [
                [
                    persist.tile([P, PW], f16, tag=f"b{k}c{c}", name=f"b{k}c{c}")
                    for c in range(NCHUNK)
                ]
                for k in range(2)
            ]

            # ones weight for the aug matmuls is generated on-chip
            nc.vector.memset(ones[:], 1.0)

            # input loads, first-needed first, FIFO on the sync ring
            def load_chunk(c):
                for k in range(2):
                    nc.sync.dma_start(
                        b[k][c][:], b_feat[k * P : (k + 1) * P, c * PW : (c + 1) * PW]
                    )

            nc.sync.dma_start(a0[:], a_feat[0:P, :])
            nc.sync.dma_start(a1[:], a_feat[P : 2 * P, :])
            load_chunk(0)
            for g in range(4):
                nc.sync.dma_start(baug[32 * g : 32 * g + AUG, :], b_aug[:])
            nc.sync.dma_start(sq1s[:], sq1t[:])
            for c in range(1, NCHUNK):
                load_chunk(c)

            a_ops = (a0, a1)

            for m in range(M1 // P):  # 8 output-row blocks
                ms = slice(m * P, (m + 1) * P)
                for t in range(NCHUNK):  # 4 psum tiles of [128, 2048]
                    ost = ostage.tile([P, PW], f16, tag="orow")
                    pst = pspool.tile([P, PW], f32, tag="ps")
                    # cross term: two K=128 weight phases, 4 matmuls each
                    for k in range(2):
                        for j in range(PW // NT):
                            nc.tensor.matmul(
                                pst[:, j * NT : (j + 1) * NT],
                                a_ops[k][:, ms],
                                b[k][t][:, j * NT : (j + 1) * NT],
                                start=(k == 0),
                                stop=False,
                            )
                    # sq2 via K=2 aug matmuls, 4-packed on row strips
                    for j in range(PW // NT):
                        gp = 32 * j
                        nc.tensor.matmul(
                            pst[:, j * NT : (j + 1) * NT],
                            ones[gp : gp + AUG, 0:P],
                            baug[gp : gp + AUG, t * PW + j * NT : t * PW + (j + 1) * NT],
                            start=False,
                            stop=True,
                            tile_position=(gp, 0),
                        )
                    # evict: out = sqrt(psum + sq1[row]) in one ACT, fp16 out
                    nc.scalar.activation(
                        ost[:],
                        pst[:],
                        Sqrt,
                        bias=sq1s[:, m : m + 1],
                    )
                    # cast-DMA fp16 -> fp32 straight to DRAM
                    nc.gpsimd.dma_start(
                        out[ms, t * PW : (t + 1) * PW], ost[:]
                    )

    ndrop = _dedupe_ldweights(nc)
    assert ndrop >= 100, f"LDW dedupe removed only {ndrop}"
    nc.finalize()
    return nc


def _prep_inputs(x1, x2):
    """Host-side sharding prep: transpose, fp16 casts, hi/lo norm split."""
    x1 = np.asarray(x1, dtype=np.float32)
    x2 = np.asarray(x2, dtype=np.float32)

    sq1 = (x1.astype(np.float64) ** 2).sum(axis=1)
    sq2 = (x2.astype(np.float64) ** 2).sum(axis=1)

    a_feat_all = np.ascontiguousarray((-2.0 * x1).T.astype(np.float16))  # [D, N1]
    b_feat = np.ascontiguousarray(x2.T.astype(np.float16))  # [D, N2]

    sq2_hi = sq2.astype(np.float16)
    sq2_lo = (sq2 - sq2_hi.astype(np.float64)).astype(np.float16)
    b_aug = np.ascontiguousarray(np.stack([sq2_hi, sq2_lo], axis=0))  # [2, N2]

    # per-partition bias layout: sq1t[p, m] = sq1[m*128 + p]
    sq1t_all = np.ascontiguousarray(
        sq1.astype(np.float32).reshape(N1 // P, P).T
    )  # [P, N1//P]

    in_maps = []
    for c in range(NCORES):
        sl = slice(c * M1, (c + 1) * M1)
        msl = slice(c * (M1 // P), (c + 1) * (M1 // P))
        in_maps.append(
            {
                "a_feat": np.ascontiguousarray(a_feat_all[:, sl]),
                "sq1t": np.ascontiguousarray(sq1t_all[:, msl]),
                "b_feat": b_feat,
                "b_aug": b_aug,
            }
        )
    return in_maps


def _run(in_maps, trace=False):
    global _built
    from concourse.bass_utils import run_bass_kernel_spmd

    if _built is None:
        _built = _build_nc()
    return run_bass_kernel_spmd(_built, in_maps, list(range(NCORES)), trace=trace)


def kernel(x1, x2):
    in_maps = _prep_inputs(x1, x2)
    res = _run(in_maps, trace=False)
    return np.concatenate([res.results[c]["out"] for c in range(NCORES)], axis=0)
